# revision 1
# baseline (speedup 1.0000x reference)
"""EnhancedGNNTransformer Trainium2 kernel (8 NeuronCores, SPMD).

Self-contained: takes FULL inputs (as reference.setup_inputs()), returns the
FULL [G, 2] output. Data-parallel over graphs: nodes laid out in padded
per-graph slots, edges partitioned by dst core and dst-sorted into a shared
static chunk grid; gathers via indirect DMA; segment softmax/sums via
scaled-one-hot matmuls accumulated in PSUM; 4 AllGathers share cross-core
tables. All matmuls in float32r (TF32-like).
"""
import sys, os
for _p in ('/opt/trn_rl_repo', '/root/.axon_site/_ro/trn_rl_repo'):
    if os.path.isdir(_p) and _p not in sys.path:
        sys.path.insert(0, _p)

import numpy as np

N, E, DIN, HID, G, L = 50000, 200000, 128, 128, 64, 6
H1, H2 = 8, 4
EPS_LN = 1e-5
EPS_BN = 1e-5
NCORES = 8
NG = G // NCORES
PBLK = 128
TTILE = 512

_cache = {}


# ----------------------------------------------------------------------------
# Host preprocessing
# ----------------------------------------------------------------------------

def preprocess(inputs):
    batch = np.asarray(inputs['batch']).astype(np.int64)
    ei = np.asarray(inputs['edge_index']).astype(np.int64)
    x = np.asarray(inputs['x']).astype(np.float32)

    cnt = np.bincount(batch, minlength=G)
    gstart = np.zeros(G + 1, np.int64)
    np.cumsum(cnt, out=gstart[1:])
    PADN = int(-(-cnt.max() // PBLK) * PBLK)
    NP = NG * PADN
    NBLK = NP // PBLK
    NPT = NCORES * NP

    n_ar = np.arange(N, dtype=np.int64)
    g_of = batch
    pad_of = (g_of // NG) * NP + (g_of % NG) * PADN + (n_ar - gstart[g_of])

    src_all = np.concatenate([ei[0], n_ar])
    dst_all = np.concatenate([ei[1], n_ar])
    deg = np.bincount(dst_all, minlength=N).astype(np.float64)
    dinv = 1.0 / np.sqrt(np.maximum(deg, 1e-12))
    norm_all = (dinv[src_all] * dinv[dst_all]).astype(np.float32)
    ps_all = pad_of[src_all]
    pd_all = pad_of[dst_all]
    ecore = pd_all // NP

    per_core = []
    cnt_cb = np.zeros((NCORES, NBLK), np.int64)
    for c in range(NCORES):
        m = ecore == c
        ps_c, pdl_c, nrm_c = ps_all[m], pd_all[m] - c * NP, norm_all[m]
        order = np.argsort(pdl_c, kind='stable')
        ps_c, pdl_c, nrm_c = ps_c[order], pdl_c[order], nrm_c[order]
        cnt_cb[c] = np.bincount(pdl_c // PBLK, minlength=NBLK)
        per_core.append((ps_c, pdl_c, nrm_c))
    M_b = (-(-cnt_cb // PBLK)).max(axis=0)
    NCHK = int(M_b.sum())
    blk_chunk0 = np.zeros(NBLK + 1, np.int64)
    np.cumsum(M_b, out=blk_chunk0[1:])

    orig_of = np.zeros(NPT, np.int64)
    orig_of[pad_of] = n_ar

    cores = []
    for c in range(NCORES):
        ps_c, pdl_c, nrm_c = per_core[c]
        nslots = NCHK * PBLK
        e_srcg = np.zeros(nslots, np.int64)
        e_dstp = np.zeros(nslots, np.int64)
        e_dloc = np.full(nslots, -1.0, np.float32)
        e_norm = np.zeros(nslots, np.float32)
        valid = np.zeros(nslots, bool)
        bstart = np.searchsorted(pdl_c // PBLK, np.arange(NBLK + 1))
        for b in range(NBLK):
            lo, hi = int(bstart[b]), int(bstart[b + 1])
            s0 = int(blk_chunk0[b]) * PBLK
            sl = slice(s0, s0 + hi - lo)
            e_srcg[sl] = ps_c[lo:hi]
            e_dstp[sl] = pdl_c[lo:hi]
            e_dloc[sl] = (pdl_c[lo:hi] % PBLK).astype(np.float32)
            e_norm[sl] = nrm_c[lo:hi]
            valid[sl] = True

        used = np.unique(e_srcg[valid])
        NUP = int(-(-max(len(used), 1) // PBLK) * PBLK)
        uid_map = np.zeros(NPT, np.int64)
        uid_map[used] = np.arange(len(used))
        e_srcu = np.where(valid, uid_map[e_srcg], 0)
        dst_glob = np.where(valid, e_dstp + c * NP, used[0])
        e_dstu = np.where(valid, uid_map[dst_glob], 0)

        xT = np.zeros((DIN, NUP), np.float32)
        xT[:, :len(used)] = x[orig_of[used]].T

        mask = np.zeros(NP, np.float32)
        invc = np.zeros(NG, np.float32)
        for gg in range(NG):
            cc = int(cnt[c * NG + gg])
            mask[gg * PADN: gg * PADN + cc] = 1.0
            invc[gg] = 1.0 / max(float(cc), 1.0)

        r2 = lambda a, dt: np.ascontiguousarray(
            a.reshape(NCHK, PBLK).T).astype(dt)
        cores.append(dict(
            xT=xT, NUP=NUP,
            ESRCU=r2(e_srcu, np.int32), EDSTU=r2(e_dstu, np.int32),
            ESRCG=r2(e_srcg, np.int32),
            EDSTG=r2(np.where(valid, e_dstp + c * NP, 0), np.int32),
            EDSTP=r2(np.where(valid, e_dstp, 0), np.int32),
            EDLOC=r2(e_dloc, np.float32), ENORM=r2(e_norm, np.float32),
            MASK=np.broadcast_to(mask, (PBLK, NP)).copy(),
            INVC=np.broadcast_to(invc, (PBLK, NG)).copy(),
        ))

    NUPmax = max(cr['NUP'] for cr in cores)
    for cr in cores:
        if cr['xT'].shape[1] < NUPmax:
            t = np.zeros((DIN, NUPmax), np.float32)
            t[:, :cr['xT'].shape[1]] = cr['xT']
            cr['xT'] = t
        cr['NUP'] = NUPmax

    meta = dict(PADN=PADN, NP=NP, NBLK=NBLK, NPT=NPT, NUP=NUPmax,
                NCHK=NCHK, M_b=[int(v) for v in M_b])
    return meta, cores


def fold_weights(inputs):
    f64 = lambda a: np.asarray(a, np.float64)
    W1, W2 = f64(inputs['W1']), f64(inputs['W2'])
    as1, ad1 = f64(inputs['a_src1']), f64(inputs['a_dst1'])
    as2, ad2 = f64(inputs['a_src2']), f64(inputs['a_dst2'])
    W1r = W1.reshape(DIN, H1, HID)
    W1cat = np.concatenate(
        [W1, np.einsum('dhc,hc->dh', W1r, as1),
         np.einsum('dhc,hc->dh', W1r, ad1)], axis=1).astype(np.float32)
    W2r = W2.reshape(H1 * HID, H2, HID)
    W2cat = np.concatenate(
        [W2, np.einsum('dhc,hc->dh', W2r, as2),
         np.einsum('dhc,hc->dh', W2r, ad2)], axis=1).astype(np.float32)
    wv, wo = f64(inputs['wv']), f64(inputs['wo'])
    bv, bo = f64(inputs['bv']), f64(inputs['bo'])
    bn_s = 1.0 / np.sqrt(1.0 + EPS_BN)
    g1, b1h = f64(inputs['bn1_g']) * bn_s, f64(inputs['bn1_b'])
    g2, b2h = f64(inputs['bn2_g']) * bn_s, f64(inputs['bn2_b'])
    fc1w, fc1b = f64(inputs['fc1_w']), f64(inputs['fc1_b'])
    fc2w, fc2b = f64(inputs['fc2_w']), f64(inputs['fc2_b'])
    ln = {k: np.asarray(inputs[k], np.float32)
          for k in ('ln1g', 'ln1b', 'ln2g', 'ln2b')}
    out = dict(
        W1cat=W1cat,
        W2cat=np.ascontiguousarray(
            W2cat.reshape(H1, HID, H2 * HID + 2 * H2)),
        W3cat=np.ascontiguousarray(
            np.asarray(inputs['W3'], np.float32).reshape(H2, HID, HID)),
        b1t=np.ascontiguousarray(
            np.asarray(inputs['b1'], np.float32).reshape(H1, HID).T),
        b2t=np.ascontiguousarray(
            np.asarray(inputs['b2'], np.float32).reshape(H2, HID).T),
        b3c=np.asarray(inputs['b3'], np.float32).reshape(HID, 1),
        WVO=np.einsum('lcd,lde->lce', wv, wo).astype(np.float32),
        BVO=np.ascontiguousarray(
            (np.einsum('ld,lde->le', bv, wo) + bo).astype(np.float32).T),
        WF1=np.asarray(inputs['wf1'], np.float32),
        BF1=np.ascontiguousarray(
            np.asarray(inputs['bf1'], np.float32).reshape(L, 4, HID).transpose(2, 0, 1)),
        WF2=np.ascontiguousarray(
            np.asarray(inputs['wf2'], np.float32).reshape(L, 4, HID, HID)),
        BF2=np.ascontiguousarray(np.asarray(inputs['bf2'], np.float32).T),
        LN1G=np.ascontiguousarray(ln['ln1g'].T),
        LN1B=np.ascontiguousarray(ln['ln1b'].T),
        LN2G=np.ascontiguousarray(ln['ln2g'].T),
        LN2B=np.ascontiguousarray(ln['ln2b'].T),
        FC1W=np.ascontiguousarray(
            (g1[:, None] * fc1w).astype(np.float32).reshape(2, HID, HID // 2)),
        FC1B=(b1h @ fc1w + fc1b).astype(np.float32).reshape(1, HID // 2),
        FC2W=(g2[:, None] * fc2w).astype(np.float32),
        FC2B=(b2h @ fc2w + fc2b).astype(np.float32).reshape(2, 1),
    )
    out['ln_trivial'] = bool(
        np.all(ln['ln1g'] == 1) and np.all(ln['ln2g'] == 1)
        and np.all(ln['ln1b'] == 0) and np.all(ln['ln2b'] == 0))
    return out


# ----------------------------------------------------------------------------
# Bass program
# ----------------------------------------------------------------------------

def build(meta, ln_trivial, dbg=False):
    import concourse.bass as bass
    import concourse.bacc as bacc
    import concourse.tile as tile
    import concourse.mybir as mybir
    from concourse.masks import make_identity
    from contextlib import ExitStack

    f32 = mybir.dt.float32
    f32r = mybir.dt.float32r
    i32 = mybir.dt.int32
    AF = mybir.ActivationFunctionType
    OP = mybir.AluOpType

    NP, NBLK, NPT = meta['NP'], meta['NBLK'], meta['NPT']
    NUP, NCHK, M_b = meta['NUP'], meta['NCHK'], meta['M_b']
    PADN = meta['PADN']
    NT = NP // TTILE

    nc = bacc.Bacc("TRN2", target_bir_lowering=False, debug=False,
                   num_devices=NCORES)

    def din(name, shape, dt):
        return nc.dram_tensor(name, shape, dt, kind="ExternalInput")

    xT = din("xT", [DIN, NUP], f32r)
    W1cat = din("W1cat", [DIN, H1 * HID + 2 * H1], f32r)
    W2cat = din("W2cat", [H1, HID, H2 * HID + 2 * H2], f32r)
    W3cat = din("W3cat", [H2, HID, HID], f32r)
    b1t = din("b1t", [HID, H1], f32)
    b2t = din("b2t", [HID, H2], f32)
    b3c = din("b3c", [HID, 1], f32)
    IOTA = din("IOTA", [PBLK, H1 * PBLK], f32)
    ESRCU = din("ESRCU", [PBLK, NCHK], i32)
    EDSTU = din("EDSTU", [PBLK, NCHK], i32)
    ESRCG = din("ESRCG", [PBLK, NCHK], i32)
    EDSTG = din("EDSTG", [PBLK, NCHK], i32)
    EDSTP = din("EDSTP", [PBLK, NCHK], i32)
    EDLOC = din("EDLOC", [PBLK, NCHK], f32)
    ENORM = din("ENORM", [PBLK, NCHK], f32)
    WVO = din("WVO", [L, HID, HID], f32r)
    BVO = din("BVO", [HID, L], f32)
    WF1 = din("WF1", [L, HID, 4 * HID], f32r)
    BF1 = din("BF1", [HID, L, 4], f32)
    WF2 = din("WF2", [L, 4, HID, HID], f32r)
    BF2 = din("BF2", [HID, L], f32)
    LN1G = din("LN1G", [HID, L], f32)
    LN1B = din("LN1B", [HID, L], f32)
    LN2G = din("LN2G", [HID, L], f32)
    LN2B = din("LN2B", [HID, L], f32)
    MASK = din("MASK", [PBLK, NP], f32)
    INVC = din("INVC", [PBLK, NG], f32)
    FC1W = din("FC1W", [2, HID, HID // 2], f32r)
    FC1B = din("FC1B", [1, HID // 2], f32r)
    FC2W = din("FC2W", [HID // 2, 2], f32r)
    FC2B = din("FC2B", [2, 1], f32)
    ONESC = din("ONESC", [PBLK, 1], f32r)   # 1/128
    ONESR = din("ONESR", [1, PBLK], f32r)   # 1.0

    OUT = nc.dram_tensor("OUT", [2, G], f32, kind="ExternalOutput")
    if dbg:
        DBG_R81 = nc.dram_tensor("DBG_R81", [NP, H1], f32, kind="ExternalOutput")
        DBG_T2 = nc.dram_tensor("DBG_T2", [NP, H2 * HID], f32, kind="ExternalOutput")
        DBG_TH2 = nc.dram_tensor("DBG_TH2", [NP, 2 * H2], f32, kind="ExternalOutput")
        DBG_T3 = nc.dram_tensor("DBG_T3", [NP, HID], f32, kind="ExternalOutput")
        DBG_H = nc.dram_tensor("DBG_H", [PBLK, NP], f32, kind="ExternalOutput")
        DBG_HF = nc.dram_tensor("DBG_HF", [PBLK, NP], f32, kind="ExternalOutput")
        DBG_T1 = nc.dram_tensor("DBG_T1", [PBLK, H1 * HID + 16], f32, kind="ExternalOutput")
        DBG_EX = nc.dram_tensor("DBG_EX", [PBLK * 4, H1], f32, kind="ExternalOutput")
        DBG_HC = nc.dram_tensor("DBG_HC", [PBLK, 2, NCORES, NG], f32, kind="ExternalOutput")

    T1 = nc.dram_tensor("T1", [NUP, H1 * HID], f32r)
    THS1 = nc.dram_tensor("THS1", [NUP, H1], f32)
    THD1 = nc.dram_tensor("THD1", [NUP, H1], f32)
    EX1 = nc.dram_tensor("EX1", [NCHK * PBLK, H1], f32r)
    R81 = nc.dram_tensor("R81", [NP, H1], f32)
    EX2 = nc.dram_tensor("EX2", [NCHK * PBLK, H2], f32r)
    R82 = nc.dram_tensor("R82", [NP, H2], f32)
    STATDR = nc.dram_tensor("STATDR", [NP // TTILE, 2, TTILE], f32)
    VDR = nc.dram_tensor("VDR", [NP // TTILE, TTILE], f32)

    def chunk_iter():
        q = 0
        for b in range(NBLK):
            for m in range(int(M_b[b])):
                yield b, m, int(M_b[b]), q
                q += 1

    with ExitStack() as ctx:
        tc = ctx.enter_context(tile.TileContext(nc))
        const = ctx.enter_context(tc.tile_pool(name="const", bufs=1))
        dram = ctx.enter_context(tc.tile_pool(name="dram", bufs=1, space="DRAM"))
        strip = ctx.enter_context(tc.tile_pool(name="strip", bufs=1))

        T2F = dram.tile([NPT, H2 * HID], f32r)
        THS2F = dram.tile([NPT, H2], f32)
        THD2F = dram.tile([NPT, H2], f32)
        T3F = dram.tile([NPT, HID], f32r)
        T2OWN = dram.tile([NP, H2 * HID], f32r)
        THS2OWN = dram.tile([NP, H2], f32)
        THD2OWN = dram.tile([NP, H2], f32)
        T3OWN = dram.tile([NP, HID], f32r)
        HCOWN = dram.tile([PBLK, 2 * NG], f32r)
        HCF = dram.tile([NCORES * PBLK, 2 * NG], f32r)

        def cload(name, dram_t, shape, dt):
            t = const.tile(shape, dt, tag=name)
            nc.sync.dma_start(out=t[:], in_=dram_t[:])
            return t

        iota_t = cload("iota", IOTA, [PBLK, H1 * PBLK], f32)
        w1c = cload("w1c", W1cat, [DIN, H1 * HID + 2 * H1], f32r)
        esrcu = cload("esrcu", ESRCU, [PBLK, NCHK], i32)
        edstu = cload("edstu", EDSTU, [PBLK, NCHK], i32)
        esrcg = cload("esrcg", ESRCG, [PBLK, NCHK], i32)
        edstg = cload("edstg", EDSTG, [PBLK, NCHK], i32)
        edstp = cload("edstp", EDSTP, [PBLK, NCHK], i32)
        edloc = cload("edloc", EDLOC, [PBLK, NCHK], f32)
        enorm = cload("enorm", ENORM, [PBLK, NCHK], f32)
        b1ts = cload("b1ts", b1t, [HID, H1], f32)
        b2ts = cload("b2ts", b2t, [HID, H2], f32)
        b3cs = cload("b3cs", b3c, [HID, 1], f32)
        ones128r = cload("ones128r", ONESC, [PBLK, 1], f32r)
        ones1r = cload("ones1r", ONESR, [1, PBLK], f32r)
        epsc = const.tile([PBLK, 1], f32, tag="epsc")
        nc.vector.memset(epsc[:], EPS_LN)
        bvo_s = cload("bvo", BVO, [HID, L], f32)
        bf1_s = cload("bf1", BF1, [HID, L, 4], f32)
        bf2_s = cload("bf2", BF2, [HID, L], f32)
        ln1g_s = cload("ln1g", LN1G, [HID, L], f32)
        ln1b_s = cload("ln1b", LN1B, [HID, L], f32)
        ln2g_s = cload("ln2g", LN2G, [HID, L], f32)
        ln2b_s = cload("ln2b", LN2B, [HID, L], f32)

        wvo_s, wf1_s, wf2_s = [], [], []
        for l in range(L):
            t = const.tile([HID, HID], f32r, tag=f"wvo{l}")
            nc.sync.dma_start(out=t[:], in_=WVO[l, :, :])
            wvo_s.append(t)
            t = const.tile([HID, 4 * HID], f32r, tag=f"wf1{l}")
            nc.sync.dma_start(out=t[:], in_=WF1[l, :, :])
            wf1_s.append(t)
            ks = []
            for k in range(4):
                t = const.tile([HID, HID], f32r, tag=f"wf2{l}_{k}")
                nc.sync.dma_start(out=t[:], in_=WF2[l, k, :, :])
                ks.append(t)
            wf2_s.append(ks)

        w2c = []
        for h in range(H1):
            t = const.tile([HID, H2 * HID + 2 * H2], f32r, tag=f"w2c{h}")
            nc.sync.dma_start(out=t[:], in_=W2cat[h, :, :])
            w2c.append(t)
        w3c = []
        for h in range(H2):
            t = const.tile([HID, HID], f32r, tag=f"w3c{h}")
            nc.sync.dma_start(out=t[:], in_=W3cat[h, :, :])
            w3c.append(t)

        HSTRIP = strip.tile([PBLK, NP], f32r, tag="H")

        # ---------------- Stage A ----------------
        with tc.tile_pool(name="sta_sb", bufs=3) as sb, \
             tc.tile_pool(name="sta_ps", bufs=2, space="PSUM") as ps:
            for t in range(NUP // PBLK):
                p = ps.tile([PBLK, 3, 512], f32, tag="pa")
                xts = sb.tile([DIN, PBLK], f32r, tag="xts")
                nc.sync.dma_start(out=xts[:], in_=xT[:, t * PBLK:(t + 1) * PBLK])
                nc.tensor.matmul(out=p[:, 0, :], lhsT=xts[:], rhs=w1c[:, 0:512],
                                 start=True, stop=True)
                nc.tensor.matmul(out=p[:, 1, :], lhsT=xts[:], rhs=w1c[:, 512:1024],
                                 start=True, stop=True)
                nc.tensor.matmul(out=p[:, 2, 0:16], lhsT=xts[:], rhs=w1c[:, 1024:1040],
                                 start=True, stop=True)
                row = sb.tile([PBLK, H1 * HID], f32r, tag="row")
                nc.vector.tensor_copy(out=row[:, 0:512], in_=p[:, 0, :])
                nc.scalar.copy(out=row[:, 512:1024], in_=p[:, 1, :])
                sm = sb.tile([PBLK, 16], f32, tag="sm")
                nc.vector.tensor_copy(out=sm[:], in_=p[:, 2, 0:16])
                nc.sync.dma_start(out=T1[t * PBLK:(t + 1) * PBLK, :], in_=row[:])
                nc.sync.dma_start(out=THS1[t * PBLK:(t + 1) * PBLK, :], in_=sm[:, 0:8])
                nc.sync.dma_start(out=THD1[t * PBLK:(t + 1) * PBLK, :], in_=sm[:, 8:16])
                if dbg and t == 0:
                    nc.sync.dma_start(out=DBG_T1[:, 0:H1 * HID],
                                      in_=row[:].bitcast(f32))
                    nc.sync.dma_start(out=DBG_T1[:, H1 * HID:], in_=sm[:])

        # ---------------- GAT pass 1 ----------------
        def gat_pass1(H, THS, THD, src_idx, dst_idx, EXT, R8T):
            with tc.tile_pool(name="p1_sb", bufs=4) as sb, \
                 tc.tile_pool(name="p1_ps", bufs=2, space="PSUM") as ps:
                ps_s = None
                for b, m, Mb, q in chunk_iter():
                    if m == 0:
                        ps_s = ps.tile([PBLK, H], f32, tag="ps_s")
                    hs = sb.tile([PBLK, H], f32, tag="hs")
                    nc.gpsimd.indirect_dma_start(
                        out=hs[:], out_offset=None, in_=THS[:],
                        in_offset=bass.IndirectOffsetOnAxis(ap=src_idx[:, q:q + 1], axis=0))
                    hd = sb.tile([PBLK, H], f32, tag="hd")
                    nc.gpsimd.indirect_dma_start(
                        out=hd[:], out_offset=None, in_=THD[:],
                        in_offset=bass.IndirectOffsetOnAxis(ap=dst_idx[:, q:q + 1], axis=0))
                    t8 = sb.tile([PBLK, H], f32, tag="t8")
                    nc.vector.tensor_add(out=t8[:], in0=hs[:], in1=hd[:])
                    lk = sb.tile([PBLK, H], f32, tag="lk")
                    nc.vector.scalar_tensor_tensor(
                        out=lk[:], in0=t8[:], scalar=0.2, in1=t8[:],
                        op0=OP.mult, op1=OP.max)
                    ex = sb.tile([PBLK, H], f32r, tag="ex")
                    nc.scalar.activation(out=ex[:], in_=lk[:], func=AF.Exp)
                    nc.sync.dma_start(out=EXT[q * PBLK:(q + 1) * PBLK, :], in_=ex[:])
                    if dbg and H == H1 and q < 4:
                        nc.sync.dma_start(
                            out=DBG_EX[q * PBLK:(q + 1) * PBLK, :],
                            in_=ex[:].bitcast(f32))
                    oh = sb.tile([PBLK, PBLK], f32r, tag="oh")
                    nc.vector.tensor_scalar(
                        out=oh[:], in0=iota_t[:, 0:PBLK], scalar1=edloc[:, q:q + 1],
                        scalar2=None, op0=OP.is_equal)
                    nc.tensor.matmul(out=ps_s[:], lhsT=oh[:], rhs=ex[:],
                                     start=(m == 0), stop=(m == Mb - 1))
                    if m == Mb - 1:
                        s_sb = sb.tile([PBLK, H], f32, tag="s_sb")
                        nc.vector.tensor_scalar(
                            out=s_sb[:], in0=ps_s[:], scalar1=1e-16,
                            scalar2=None, op0=OP.add)
                        r8 = sb.tile([PBLK, H], f32, tag="r8")
                        nc.vector.reciprocal(out=r8[:], in_=s_sb[:])
                        nc.sync.dma_start(out=R8T[b * PBLK:(b + 1) * PBLK, :], in_=r8[:])
                        if dbg and H == H1:
                            nc.sync.dma_start(
                                out=DBG_R81[b * PBLK:(b + 1) * PBLK, :], in_=r8[:])

        # ---------------- GAT pass 2 (+fused table matmul) ----------------
        def gat_pass2(H, T, src_idx, EXT, R8T, btile, consumer):
            W = H * HID
            nb = 4   # heads per PSUM bank (512 f32 / 128) -> start flag per bank
            with tc.tile_pool(name="p2_sb", bufs=3) as sb, \
                 tc.tile_pool(name="p2_ps", bufs=2, space="PSUM") as ps:
                ps_g = None
                for b, m, Mb, q in chunk_iter():
                    if m == 0:
                        ps_g = ps.tile([PBLK, H, PBLK], f32, tag="ps_g")
                    g = sb.tile([PBLK, W], f32r, tag="g")
                    nc.gpsimd.indirect_dma_start(
                        out=g[:], out_offset=None, in_=T[:],
                        in_offset=bass.IndirectOffsetOnAxis(ap=src_idx[:, q:q + 1], axis=0))
                    exl = sb.tile([PBLK, H], f32r, tag="exl")
                    nc.sync.dma_start(out=exl[:], in_=EXT[q * PBLK:(q + 1) * PBLK, :])
                    rpe = sb.tile([PBLK, H], f32, tag="rpe")
                    nc.gpsimd.indirect_dma_start(
                        out=rpe[:], out_offset=None, in_=R8T[:],
                        in_offset=bass.IndirectOffsetOnAxis(ap=edstp[:, q:q + 1], axis=0))
                    al = sb.tile([PBLK, H], f32, tag="al")
                    nc.vector.tensor_mul(out=al[:], in0=exl[:], in1=rpe[:])
                    mt = sb.tile([PBLK, H, PBLK], f32r, tag="mt")
                    nc.vector.scalar_tensor_tensor(
                        out=mt[:],
                        in0=iota_t[:, 0:W].rearrange("p (h i) -> p h i", h=H),
                        scalar=edloc[:, q:q + 1],
                        in1=al[:, :, None].to_broadcast([PBLK, H, PBLK]),
                        op0=OP.is_equal, op1=OP.mult)
                    for h in range(H):
                        nc.tensor.matmul(
                            out=ps_g[:, h, :],
                            lhsT=g[:, h * HID:(h + 1) * HID],
                            rhs=mt[:, h, :],
                            start=(m == 0 and h % nb == 0),
                            stop=(m == Mb - 1 and h % nb == nb - 1),
                            skip_group_check=True)
                    if m == Mb - 1:
                        xb = sb.tile([PBLK, H, PBLK], f32, tag="xb")
                        nc.vector.tensor_tensor(
                            out=xb[:], in0=ps_g[:],
                            in1=btile[:, :, None].to_broadcast([PBLK, H, PBLK]),
                            op=OP.add)
                        mm_ = sb.tile([PBLK, H, PBLK], f32, tag="mm_")
                        nc.vector.tensor_scalar(
                            out=mm_[:], in0=xb[:], scalar1=0.0, scalar2=None,
                            op0=OP.min)
                        ee = sb.tile([PBLK, H, PBLK], f32, tag="ee")
                        nc.scalar.activation(out=ee[:], in_=mm_[:], func=AF.Exp)
                        hfm = sb.tile([PBLK, H, PBLK], f32r, tag="hfm")
                        nc.vector.scalar_tensor_tensor(
                            out=hfm[:], in0=ee[:], scalar=-1.0, in1=xb[:],
                            op0=OP.add, op1=OP.max)
                        consumer(b, hfm, sb, ps)

        def stageB(b, hfm, sb, ps):
            pb = ps.tile([PBLK, 2, 512], f32, tag="pb")
            for h in range(H1):
                nc.tensor.matmul(out=pb[:, 0, :], lhsT=hfm[:, h, :],
                                 rhs=w2c[h][:, 0:512],
                                 start=(h == 0), stop=(h == H1 - 1),
                                 skip_group_check=True)
                nc.tensor.matmul(out=pb[:, 1, 0:8], lhsT=hfm[:, h, :],
                                 rhs=w2c[h][:, 512:520],
                                 start=(h == 0), stop=(h == H1 - 1),
                                 skip_group_check=True)
            t2row = sb.tile([PBLK, 512], f32r, tag="t2row")
            nc.scalar.copy(out=t2row[:], in_=pb[:, 0, :])
            sm2 = sb.tile([PBLK, 8], f32, tag="sm2")
            nc.vector.tensor_copy(out=sm2[:], in_=pb[:, 1, 0:8])
            nc.sync.dma_start(out=T2OWN[b * PBLK:(b + 1) * PBLK, :], in_=t2row[:])
            nc.sync.dma_start(out=THS2OWN[b * PBLK:(b + 1) * PBLK, :], in_=sm2[:, 0:4])
            nc.sync.dma_start(out=THD2OWN[b * PBLK:(b + 1) * PBLK, :], in_=sm2[:, 4:8])
            if dbg:
                nc.sync.dma_start(out=DBG_T2[b * PBLK:(b + 1) * PBLK, :],
                                  in_=t2row[:].bitcast(f32))
                nc.sync.dma_start(out=DBG_TH2[b * PBLK:(b + 1) * PBLK, :],
                                  in_=sm2[:])

        def stageC(b, hfm, sb, ps):
            pc = ps.tile([PBLK, HID], f32, tag="pc")
            for h in range(H2):
                nc.tensor.matmul(out=pc[:], lhsT=hfm[:, h, :], rhs=w3c[h][:],
                                 start=(h == 0), stop=(h == H2 - 1),
                                 skip_group_check=True)
            t3row = sb.tile([PBLK, HID], f32r, tag="t3row")
            nc.scalar.copy(out=t3row[:], in_=pc[:])
            nc.sync.dma_start(out=T3OWN[b * PBLK:(b + 1) * PBLK, :], in_=t3row[:])
            if dbg:
                nc.sync.dma_start(out=DBG_T3[b * PBLK:(b + 1) * PBLK, :],
                                  in_=t3row[:].bitcast(f32))

        gat_pass1(H1, THS1, THD1, esrcu, edstu, EX1, R81)
        gat_pass2(H1, T1, esrcu, EX1, R81, b1ts, stageB)

        rg = [list(range(NCORES))]
        nc.gpsimd.collective_compute("AllGather", OP.bypass, replica_groups=rg,
                                     ins=[T2OWN.opt()], outs=[T2F.opt()])
        nc.gpsimd.collective_compute("AllGather", OP.bypass, replica_groups=rg,
                                     ins=[THS2OWN.opt()], outs=[THS2F.opt()])
        nc.gpsimd.collective_compute("AllGather", OP.bypass, replica_groups=rg,
                                     ins=[THD2OWN.opt()], outs=[THD2F.opt()])

        gat_pass1(H2, THS2F, THD2F, esrcg, edstg, EX2, R82)
        gat_pass2(H2, T2F, esrcg, EX2, R82, b2ts, stageC)

        nc.gpsimd.collective_compute("AllGather", OP.bypass, replica_groups=rg,
                                     ins=[T3OWN.opt()], outs=[T3F.opt()])

        # ---------------- GCN ----------------
        with tc.tile_pool(name="gc_sb", bufs=4) as sb, \
             tc.tile_pool(name="gc_ps", bufs=2, space="PSUM") as ps:
            ps_f = None
            for b, m, Mb, q in chunk_iter():
                if m == 0:
                    ps_f = ps.tile([PBLK, HID], f32, tag="ps_f")
                g3 = sb.tile([PBLK, HID], f32r, tag="g3")
                nc.gpsimd.indirect_dma_start(
                    out=g3[:], out_offset=None, in_=T3F[:],
                    in_offset=bass.IndirectOffsetOnAxis(ap=esrcg[:, q:q + 1], axis=0))
                mt3 = sb.tile([PBLK, PBLK], f32r, tag="mt3")
                nc.vector.tensor_scalar(
                    out=mt3[:], in0=iota_t[:, 0:PBLK],
                    scalar1=edloc[:, q:q + 1], scalar2=enorm[:, q:q + 1],
                    op0=OP.is_equal, op1=OP.mult)
                nc.tensor.matmul(out=ps_f[:], lhsT=g3[:], rhs=mt3[:],
                                 start=(m == 0), stop=(m == Mb - 1))
                if m == Mb - 1:
                    xb3 = sb.tile([PBLK, PBLK], f32, tag="xb3")
                    nc.vector.tensor_scalar(
                        out=xb3[:], in0=ps_f[:], scalar1=b3cs[:, 0:1],
                        scalar2=None, op0=OP.add)
                    m3 = sb.tile([PBLK, PBLK], f32, tag="m3")
                    nc.vector.tensor_scalar(
                        out=m3[:], in0=xb3[:], scalar1=0.0, scalar2=None, op0=OP.min)
                    e3 = sb.tile([PBLK, PBLK], f32, tag="e3")
                    nc.scalar.activation(out=e3[:], in_=m3[:], func=AF.Exp)
                    nc.vector.scalar_tensor_tensor(
                        out=HSTRIP[:, b * PBLK:(b + 1) * PBLK],
                        in0=e3[:], scalar=-1.0, in1=xb3[:],
                        op0=OP.add, op1=OP.max)

        # ---------------- Transformer ----------------
        if dbg:
            nc.sync.dma_start(out=DBG_H[:], in_=HSTRIP[:].bitcast(f32))

        def ln_stats(ps, sb, x_ap, t):
            """x_ap: [128, TTILE] f32r; writes mu -> STATDR[t,0], var -> VDR[t]."""
            sq = sb.tile([PBLK, TTILE], f32r, tag="sq")
            nc.scalar.activation(out=sq[:], in_=x_ap, func=AF.Square)
            p_mu = ps.tile([1, TTILE], f32, tag="p_mu")
            nc.tensor.matmul(out=p_mu[:], lhsT=ones128r[:], rhs=x_ap,
                             start=True, stop=True)
            p_m2 = ps.tile([1, TTILE], f32, tag="p_m2")
            nc.tensor.matmul(out=p_m2[:], lhsT=ones128r[:], rhs=sq[:],
                             start=True, stop=True)
            mu_t = sb.tile([1, TTILE], f32, tag="mu_t")
            nc.scalar.activation(out=mu_t[:], in_=p_mu[:], func=AF.Copy)
            msq = sb.tile([1, TTILE], f32, tag="msq")
            nc.scalar.activation(out=msq[:], in_=mu_t[:], func=AF.Square)
            v_t = sb.tile([1, TTILE], f32, tag="v_t")
            nc.vector.tensor_tensor(out=v_t[:], in0=p_m2[:], in1=msq[:],
                                    op=OP.subtract)
            nc.sync.dma_start(out=STATDR[t, 0, :][None, :], in_=mu_t[:])
            nc.sync.dma_start(out=VDR[t, :][None, :], in_=v_t[:])

        def ln_rsqrt(sb):
            vd = sb.tile([NT, TTILE], f32, tag="vd")
            nc.sync.dma_start(out=vd[:], in_=VDR[:])
            sr = sb.tile([NT, TTILE], f32, tag="sr")
            nc.scalar.activation(out=sr[:], in_=vd[:], func=AF.Sqrt,
                                 bias=epsc[0:NT, :])
            rr = sb.tile([NT, TTILE], f32, tag="rr")
            nc.vector.reciprocal(out=rr[:], in_=sr[:])
            nc.sync.dma_start(out=STATDR[:, 1, :], in_=rr[:])

        def ln_apply(ps, sb, t, src_strip, dst_sl, gcol, bcol):
            mr = sb.tile([PBLK, 2, TTILE], f32, tag="mr")
            nc.gpsimd.dma_start(
                out=mr[:], in_=STATDR[t:t + 1, :, :].to_broadcast([PBLK, 2, TTILE]))
            xc = sb.tile([PBLK, TTILE], f32, tag="xc")
            nc.vector.tensor_tensor(out=xc[:], in0=src_strip,
                                    in1=mr[:, 0, :], op=OP.subtract)
            if ln_trivial:
                nc.vector.tensor_tensor(out=HSTRIP[:, dst_sl], in0=xc[:],
                                        in1=mr[:, 1, :], op=OP.mult)
            else:
                xn0 = sb.tile([PBLK, TTILE], f32, tag="xn0")
                nc.vector.tensor_tensor(out=xn0[:], in0=xc[:],
                                        in1=mr[:, 1, :], op=OP.mult)
                nc.vector.tensor_scalar(
                    out=HSTRIP[:, dst_sl], in0=xn0[:],
                    scalar1=gcol, scalar2=bcol, op0=OP.mult, op1=OP.add)

        with tc.tile_pool(name="tr_strip", bufs=1) as tsp, \
             tc.tile_pool(name="tr_sb", bufs=3) as sb, \
             tc.tile_pool(name="tr_ps", bufs=1, space="PSUM") as ps:
            X1S = tsp.tile([PBLK, NP], f32r, tag="X1")
            for l in range(L):
                for t in range(NT):
                    sl = slice(t * TTILE, (t + 1) * TTILE)
                    p_sa = ps.tile([PBLK, TTILE], f32, tag="p_sa")
                    nc.tensor.matmul(out=p_sa[:], lhsT=wvo_s[l][:],
                                     rhs=HSTRIP[:, sl], start=True, stop=True)
                    nc.vector.scalar_tensor_tensor(
                        out=X1S[:, sl], in0=p_sa[:], scalar=bvo_s[:, l:l + 1],
                        in1=HSTRIP[:, sl], op0=OP.add, op1=OP.add)
                    ln_stats(ps, sb, X1S[:, sl], t)
                ln_rsqrt(sb)

                for t in range(NT):
                    sl = slice(t * TTILE, (t + 1) * TTILE)
                    ln_apply(ps, sb, t, X1S[:, sl], sl,
                             ln1g_s[:, l:l + 1], ln1b_s[:, l:l + 1])
                    # FF
                    p_ff2 = ps.tile([PBLK, TTILE], f32, tag="p_ff2")
                    for half in range(2):
                        p_ff1 = ps.tile([PBLK, 2, TTILE], f32, tag="p_ff1")
                        for j in range(2):
                            k4 = half * 2 + j
                            nc.tensor.matmul(
                                out=p_ff1[:, j, :],
                                lhsT=wf1_s[l][:, k4 * HID:(k4 + 1) * HID],
                                rhs=HSTRIP[:, sl], start=True, stop=True)
                        for j in range(2):
                            k4 = half * 2 + j
                            gl_ = sb.tile([PBLK, TTILE], f32r, tag=f"gl{j}")
                            nc.scalar.activation(
                                out=gl_[:], in_=p_ff1[:, j, :], func=AF.Gelu,
                                bias=bf1_s[:, l, k4:k4 + 1])
                            nc.tensor.matmul(
                                out=p_ff2[:], lhsT=wf2_s[l][k4][:], rhs=gl_[:],
                                start=(k4 == 0), stop=(k4 == 3),
                                skip_group_check=True)
                    nc.vector.scalar_tensor_tensor(
                        out=X1S[:, sl], in0=p_ff2[:], scalar=bf2_s[:, l:l + 1],
                        in1=HSTRIP[:, sl], op0=OP.add, op1=OP.add)
                    ln_stats(ps, sb, X1S[:, sl], t)
                ln_rsqrt(sb)

                for t in range(NT):
                    sl = slice(t * TTILE, (t + 1) * TTILE)
                    ln_apply(ps, sb, t, X1S[:, sl], sl,
                             ln2g_s[:, l:l + 1], ln2b_s[:, l:l + 1])

        if dbg:
            nc.sync.dma_start(out=DBG_HF[:], in_=HSTRIP[:].bitcast(f32))

        # ---------------- Pooling + head ----------------
        with tc.tile_pool(name="po_sb", bufs=1) as sb, \
             tc.tile_pool(name="po_ps", bufs=1, space="PSUM") as ps:
            mask_s = sb.tile([PBLK, NP], f32, tag="mask")
            nc.sync.dma_start(out=mask_s[:], in_=MASK[:])
            invc_s = sb.tile([PBLK, NG], f32, tag="invc")
            nc.sync.dma_start(out=invc_s[:], in_=INVC[:])
            AX = mybir.AxisListType
            hmg = sb.tile([PBLK, NG], f32, tag="hmg")
            hxg = sb.tile([PBLK, NG], f32, tag="hxg")
            scr = sb.tile([PBLK, NP], f32, tag="scr")
            nc.vector.tensor_tensor(out=scr[:], in0=HSTRIP[:], in1=mask_s[:],
                                    op=OP.mult)
            for gg in range(NG):
                nc.vector.tensor_reduce(
                    out=hmg[:, gg:gg + 1],
                    in_=scr[:, gg * PADN:(gg + 1) * PADN],
                    axis=AX.X, op=OP.add)
            # masked max: h*mask + (mask-1)*1e9  (pads -> -1e9, reals exact)
            mneg = sb.tile([PBLK, NP], f32, tag="mneg")
            nc.vector.tensor_scalar(out=mneg[:], in0=mask_s[:], scalar1=-1.0,
                                    scalar2=1e9, op0=OP.add, op1=OP.mult)
            nc.vector.scalar_tensor_tensor(
                out=scr[:], in0=HSTRIP[:], scalar=1.0, in1=mask_s[:],
                op0=OP.mult, op1=OP.mult)
            nc.vector.tensor_tensor(out=scr[:], in0=scr[:], in1=mneg[:], op=OP.add)
            for gg in range(NG):
                nc.vector.tensor_reduce(
                    out=hxg[:, gg:gg + 1],
                    in_=scr[:, gg * PADN:(gg + 1) * PADN],
                    axis=AX.X, op=OP.max)
            hc = sb.tile([PBLK, 2, NG], f32r, tag="hc")
            nc.vector.tensor_tensor(out=hc[:, 0, :], in0=hmg[:], in1=invc_s[:],
                                    op=OP.mult)
            nc.vector.tensor_copy(out=hc[:, 1, :], in_=hxg[:])
            nc.sync.dma_start(out=HCOWN[:], in_=hc[:].rearrange("p a b -> p (a b)"))
            nc.gpsimd.collective_compute(
                "AllGather", OP.bypass, replica_groups=rg,
                ins=[HCOWN.opt()], outs=[HCF.opt()])
            hcall = sb.tile([PBLK, 2, NCORES, NG], f32r, tag="hcall")
            for c in range(NCORES):
                nc.sync.dma_start(
                    out=hcall[:, :, c, :],
                    in_=HCF[c * PBLK:(c + 1) * PBLK, :].rearrange(
                        "p (a b) -> p a b", a=2))
            if dbg:
                nc.sync.dma_start(out=DBG_HC[:], in_=hcall[:].bitcast(f32))
            fc1w_s = []
            for pp in range(2):
                t_ = sb.tile([HID, HID // 2], f32r, tag=f"fc1w{pp}")
                nc.sync.dma_start(out=t_[:], in_=FC1W[pp, :, :])
                fc1w_s.append(t_)
            fc1b_s = sb.tile([1, HID // 2], f32r, tag="fc1b")
            nc.sync.dma_start(out=fc1b_s[:], in_=FC1B[:])
            fc2w_s = sb.tile([HID // 2, 2], f32r, tag="fc2w")
            nc.sync.dma_start(out=fc2w_s[:], in_=FC2W[:])
            fc2b_s = sb.tile([2, 1], f32, tag="fc2b")
            nc.sync.dma_start(out=fc2b_s[:], in_=FC2B[:])

            p_h1 = ps.tile([G, HID // 2], f32, tag="p_h1")
            for pp in range(2):
                nc.tensor.matmul(out=p_h1[:], lhsT=hcall[:, pp, :, :],
                                 rhs=fc1w_s[pp][:],
                                 start=(pp == 0), stop=False,
                                 skip_group_check=True)
            nc.tensor.matmul(out=p_h1[:], lhsT=ones1r[:, 0:G], rhs=fc1b_s[:],
                             start=False, stop=True, skip_group_check=True)
            xh = sb.tile([G, HID // 2], f32, tag="xh")
            nc.vector.tensor_copy(out=xh[:], in_=p_h1[:])
            mh = sb.tile([G, HID // 2], f32, tag="mh")
            nc.vector.tensor_scalar(out=mh[:], in0=xh[:], scalar1=0.0,
                                    scalar2=None, op0=OP.min)
            eh = sb.tile([G, HID // 2], f32, tag="eh")
            nc.scalar.activation(out=eh[:], in_=mh[:], func=AF.Exp)
            hc2 = sb.tile([G, HID // 2], f32, tag="hc2")
            nc.vector.scalar_tensor_tensor(out=hc2[:], in0=eh[:], scalar=-1.0,
                                           in1=xh[:], op0=OP.add, op1=OP.max)
            ident = sb.tile([PBLK, PBLK], f32, tag="ident")
            make_identity(nc, ident[:])
            p_t = ps.tile([HID // 2, G], f32, tag="p_t")
            nc.tensor.transpose(out=p_t[:], in_=hc2[:], identity=ident[0:G, 0:G])
            hc2t = sb.tile([HID // 2, G], f32r, tag="hc2t")
            nc.scalar.copy(out=hc2t[:], in_=p_t[:])
            p_o = ps.tile([2, G], f32, tag="p_o")
            nc.tensor.matmul(out=p_o[:], lhsT=fc2w_s[:], rhs=hc2t[:],
                             start=True, stop=True)
            out_sb = sb.tile([2, G], f32, tag="out_sb")
            nc.scalar.activation(out=out_sb[:], in_=p_o[:], func=AF.Identity,
                                 bias=fc2b_s[:, 0:1])
            nc.sync.dma_start(out=OUT[:], in_=out_sb[:])

    nc.compile()
    return nc


# ----------------------------------------------------------------------------
# Driver
# ----------------------------------------------------------------------------

def make_in_maps(meta, cores, wf):
    iota = np.tile(np.arange(PBLK, dtype=np.float32), H1)[None, :].repeat(PBLK, 0)
    onesc = np.full((PBLK, 1), 1.0 / 128.0, np.float32)
    onesr = np.ones((1, PBLK), np.float32)
    in_maps = []
    for c in range(NCORES):
        cr = cores[c]
        m = dict(
            xT=cr['xT'], W1cat=wf['W1cat'], W2cat=wf['W2cat'],
            W3cat=wf['W3cat'], b1t=wf['b1t'], b2t=wf['b2t'], b3c=wf['b3c'],
            IOTA=iota, ESRCU=cr['ESRCU'], EDSTU=cr['EDSTU'],
            ESRCG=cr['ESRCG'], EDSTG=cr['EDSTG'], EDSTP=cr['EDSTP'],
            EDLOC=cr['EDLOC'], ENORM=cr['ENORM'],
            WVO=wf['WVO'], BVO=wf['BVO'], WF1=wf['WF1'], BF1=wf['BF1'],
            WF2=wf['WF2'], BF2=wf['BF2'], LN1G=wf['LN1G'], LN1B=wf['LN1B'],
            LN2G=wf['LN2G'], LN2B=wf['LN2B'],
            MASK=cr['MASK'], INVC=cr['INVC'],
            FC1W=wf['FC1W'], FC1B=wf['FC1B'], FC2W=wf['FC2W'], FC2B=wf['FC2B'],
            ONESC=onesc, ONESR=onesr,
        )
        in_maps.append(m)
    return in_maps


def _get_built(inputs):
    meta, cores = preprocess(inputs)
    wf = fold_weights(inputs)
    key = (meta['NUP'], meta['NCHK'], meta['PADN'], wf['ln_trivial'])
    if key not in _cache:
        _cache[key] = build(meta, wf['ln_trivial'])
    return _cache[key], meta, cores, wf


def kernel(**inputs):
    from concourse.bass_utils import run_bass_kernel_spmd
    nc, meta, cores, wf = _get_built(inputs)
    in_maps = make_in_maps(meta, cores, wf)
    res = run_bass_kernel_spmd(nc, in_maps, core_ids=list(range(NCORES)))
    out = res.results[0]["OUT"]          # [2, G]
    return np.ascontiguousarray(out.T.astype(np.float32))

